# revision 26
# baseline (speedup 1.0000x reference)
"""Trainium2 Bass kernel for nn_MultiHeadAttention (B=2, T=2048, C=1024, H=16).

Sharding (8 cores): data-parallel over batch (2) x tensor-parallel over head
groups (4 groups of 4 heads), Megatron-style. Each core computes

    out_partial = softmax((x_b @ Wq_g.T) (x_b @ Wk_g.T).T / sqrt(d)) @ (x_b @ Wv_g.T) @ Wp_g.T

for its 4 heads; the 4 partials per batch are summed on the host (row-parallel
c_proj unshard) - no device collective needed.

Device-side layout notes:
  - Host pre-arranges xT [C, T] and every weight shard directly into the SBUF
    tile layout, so all input DMAs are single contiguous 2D transfers.
  - scores are computed TRANSPOSED (scoresT[tk, tq] = k @ q.T) so that the
    AV matmul can consume exp(scoresT) directly as the moving operand.
  - V carries an appended ones-column (d=65) so the AV matmul also produces
    the softmax denominator (row 64 of yT) for free on the PE.
  - The attention inner loop is software-pipelined at emission level: the QK
    pair for tile tk+1 is queued on the PE *before* the AV matmuls for tile
    tk, so the PE computes scores while ACT runs the previous exp, keeping
    both engines saturated and HAM warm.
  - Softmax normalization is deferred: reciprocals (reciprocal_approx_fast on
    a packed [2,512] tile) and the broadcast matmul for block p are emitted
    inside block p+1's loop, and c_proj for a tq block is spread across the
    following block, so nothing ever head-of-line-blocks the PE queue.
"""

import numpy as np

import concourse.bass as bass
import concourse.mybir as mybir
import concourse.tile as tile
from concourse import bass_utils

F32 = mybir.dt.float32
F32R = mybir.dt.float32r
BF16 = mybir.dt.bfloat16


def legalize_waits(nc, max_waits=1):
    """Walrus codegen in this toolchain rejects instructions carrying more
    than one sync wait. Split extra waits into preceding same-engine NoOps
    at the BIR-JSON level and pin the serialized module on the nc object."""
    import json as _json
    d = _json.loads(nc.to_json_bytes())
    ctr = 0
    for fn in d.get("functions", []):
        for blk in fn.get("blocks", []) or []:
            insts = blk.get("instructions")
            if not insts:
                continue
            out = []
            for inst in insts:
                si = inst.get("sync_info")
                waits = (si or {}).get("on_wait") or []
                if len(waits) > max_waits:
                    keep, extra = waits[:max_waits], waits[max_waits:]
                    for w in extra:
                        ctr += 1
                        out.append({
                            "debug": inst.get("debug", 0),
                            "engine": inst["engine"],
                            "ins": [],
                            "outs": [],
                            "name": f"I-wsplit-{ctr}",
                            "opcode": "NoOp",
                            "sync_info": {"on_wait": [w], "on_update": []},
                        })
                    si["on_wait"] = keep
                out.append(inst)
            blk["instructions"] = out
    raw = _json.dumps(d).encode()
    nc.to_json_bytes = lambda: raw
    return nc

# Problem constants
B, T_FULL, C_FULL = 2, 2048, 1024
H_GLOBAL = 16
D = 64  # head dim
N_CORES = 8
HL = 4  # heads per core
CLOC = HL * D  # 256 local channels


def emit_mha_kernel(tc, out, xT, wqk, wv, wp, sel, ones, ident, T, C):
    """Emit the per-core MHA kernel into TileContext tc.

    out: dram [T, C] (partial output)
    xT:  dram [C, T]
    wqk: dram [128, CT*2*CLOC]  (pre-tiled: [p, ct, f] flattened)
    wv:  dram [128, CT*CLOC]
    wp:  dram [128, KK*C]
    sel: dram [2, 128]  (bc selector: row0 = 1 on cols 0:64, row1 = 1 on 64:128)
    """
    nc = tc.nc
    sel = sel.bitcast(F32R)

    CT = C // 128          # c (contraction) tiles for projections
    TT = T // 128          # t tiles
    KK = CLOC // 128       # c_loc tiles (2)
    BLK = min(1024, T)     # tq block (free-dim) size for phase A
    SUB = min(512, BLK)    # per-matmul N (<= one PSUM bank of fp32)
    NSUB = BLK // SUB
    NB = T // BLK          # tq blocks per row
    DE = D + 1             # v columns incl. ones
    QB = min(512, T)       # tq block per head in phase B
    NQB = T // QB
    NPH = NQB * (HL // 2)  # attention phases: (tb, g)

    import contextlib
    stack = contextlib.ExitStack()

    persist = stack.enter_context(tc.tile_pool(name="persist", bufs=1))
    main_ps = stack.enter_context(tc.tile_pool(name="main_ps", bufs=2, space="PSUM"))
    y_ps = stack.enter_context(tc.tile_pool(name="y_ps", bufs=4, space="PSUM"))
    exp_pool = stack.enter_context(tc.tile_pool(name="exp_pool", bufs=2))
    out_pool = stack.enter_context(tc.tile_pool(name="out_pool", bufs=2))
    small_pool = stack.enter_context(tc.tile_pool(name="small_pool", bufs=2))

    # ---- persistent SBUF tensors ----
    xt_sb = persist.tile([128, CT * T], BF16, name="xt_sb")
    wqk_sb = persist.tile([128, CT * 2 * CLOC], BF16, name="wqk_sb")
    wv_sb = persist.tile([128, CT * CLOC], BF16, name="wv_sb")
    wp_sb = persist.tile([128, KK * C], BF16, name="wp_sb")
    qk_sb = persist.tile([128, 4 * T], BF16, name="qk_sb")
    v_sb = persist.tile([128, TT * HL * DE], BF16, name="v_sb")
    vt_sb = persist.tile([128, 2 * T], BF16, name="vt_sb")
    yT_sb = persist.tile([128, KK * T], BF16, name="yT_sb")
    sel_sb = persist.tile([2, 128], F32R, name="sel_sb")
    ident_sb = persist.tile([128, 128], BF16, name="ident_sb")

    def mm(out_ap, lhsT, rhs, **kw):
        nc.tensor.matmul(out_ap, lhsT, rhs, **kw)

    # ---- warm up the ACT exp table during the input DMA window ----
    warm = small_pool.tile([128, 1], F32, name="warm")
    const0 = nc.const_aps.aps[(mybir.dt.float32, 0.0)]
    nc.scalar.activation(warm[:], const0, mybir.ActivationFunctionType.Exp)

    # ---- PE warmup: dummy matmuls with no input dependencies (const APs are
    # written in the Bass preamble), running while the input DMAs stream in.
    # Keeps the HAM clock gate at K=8/8 (2.4 GHz) so phase A starts at full
    # PE clock instead of paying the cold-throttle penalty.
    wps = y_ps.tile([128, 512], F32, name="yps")
    for _ in range(140):
        mm(wps[0:1, 0:1], const0, const0, start=True, stop=True)
    nc.vector.tensor_copy(warm[0:1, :], wps[0:1, 0:1])

    # ---- input DMAs (all contiguous 2D; wqk/xT split per c-tile so the
    # first A1 matmul only waits for ~1.3 MB) ----
    FQK = 2 * CLOC
    nc.sync.dma_start(wqk_sb[:, 0:FQK], wqk[:, 0:FQK])
    nc.sync.dma_start(xt_sb[:, 0:T], xT[0:128, :])
    for ct in range(1, CT):
        nc.sync.dma_start(
            wqk_sb[:, ct * FQK:(ct + 1) * FQK], wqk[:, ct * FQK:(ct + 1) * FQK]
        )
        nc.sync.dma_start(
            xt_sb[:, ct * T:(ct + 1) * T], xT[ct * 128:(ct + 1) * 128, :]
        )
    nc.sync.dma_start(wv_sb[:], wv[:])
    nc.sync.dma_start(wp_sb[:], wp[:])
    nc.sync.dma_start(sel_sb[:], sel[:])
    nc.sync.dma_start(ident_sb[:], ident[:])
    # Softmax-denominator ones column of v_ext (only column D of each head
    # block; strided, so phase A2's v writes carry no false WAW dependency).
    nc.sync.dma_start(
        v_sb[:].rearrange("p (t h e) -> p t h e", t=TT, h=HL)[:, :, :, D:DE],
        ones[:, 0:TT * HL].rearrange("p (t h e) -> p t h e", t=TT, h=HL),
    )

    # ---- Phase A1: qkT = (x @ [Wq|Wk].T).T  -> qk_sb [128, 4*T] ----
    # f_tile ft: 0 = q heads01, 1 = q heads23, 2 = k heads01, 3 = k heads23
    for ft in range(4):
        for tb in range(NB):
            ps = main_ps.tile([128, BLK], F32, name="ps_qkv")
            for ct in range(CT):
                lhsT = wqk_sb[:, ct * FQK + ft * 128: ct * FQK + (ft + 1) * 128]
                for sb in range(NSUB):
                    t0 = ct * T + tb * BLK + sb * SUB
                    mm(
                        ps[:, sb * SUB:(sb + 1) * SUB],
                        lhsT,
                        xt_sb[:, t0:t0 + SUB],
                        start=(ct == 0),
                        stop=(ct == CT - 1),
                    )
            nc.vector.tensor_copy(
                qk_sb[:, ft * T + tb * BLK: ft * T + (tb + 1) * BLK], ps[:]
            )

    # ---- Phase A2: vT = (x @ Wv.T).T via wv-stationary matmuls (stationary
    # reused across the whole T free dim, so no per-matmul LDWEIGHTS churn),
    # then PE-transpose 128x128 tiles into the natural [t, f] layout v_sb
    # needs for the AV stationary. ----
    for ft in range(2):
        for tb in range(NB):
            ps = main_ps.tile([128, BLK], F32, name="ps_qkv")
            for ct in range(CT):
                lhsT = wv_sb[:, ct * CLOC + ft * 128: ct * CLOC + (ft + 1) * 128]
                for sb in range(NSUB):
                    t0 = ct * T + tb * BLK + sb * SUB
                    mm(
                        ps[:, sb * SUB:(sb + 1) * SUB],
                        lhsT,
                        xt_sb[:, t0:t0 + SUB],
                        start=(ct == 0),
                        stop=(ct == CT - 1),
                    )
            nc.vector.tensor_copy(
                vt_sb[:, ft * T + tb * BLK: ft * T + (tb + 1) * BLK], ps[:]
            )
    def emit_vtrans(tt):
        tp = main_ps.tile([128, 256], BF16, name="tp", tag="ps_qkv")
        for kf in range(2):
            nc.tensor.transpose(
                tp[:, kf * 128:(kf + 1) * 128],
                vt_sb[:, kf * T + tt * 128: kf * T + (tt + 1) * 128],
                ident_sb[:],
            )
        for h in range(HL):
            kf, hh = divmod(h, 2)
            nc.vector.tensor_copy(
                v_sb[:, tt * HL * DE + h * DE: tt * HL * DE + h * DE + D],
                tp[:, kf * 128 + hh * 64: kf * 128 + hh * 64 + 64],
            )

    # v tiles 0,1 up front; the rest interleave into phase 0 (paired PSUM
    # allocs at every other tk slot, staying 2 tiles ahead of the AV stream).
    emit_vtrans(0)
    emit_vtrans(1)

    # ---- Phase B: attention, software-pipelined across (tb, g) phases ----
    # Per phase p = (tb, g): 16 tk iterations of {QK pair (row-packed, one tk
    # ahead), exp, AV pair (M=65 incl. denominator row)}. Normalization for
    # phase p-1 and c_proj for tb (p-2)//2 are spread across p's tk slots.
    ypairs = [None] * NPH          # live ypair psum tiles per phase
    scale = 1.0 / np.sqrt(D)

    def emit_qk(p, tk):
        tb, g = divmod(p, 2)
        qcol = g * T
        kcol = (2 + g) * T
        sc = main_ps.tile([128, 2 * QB], F32, name="sc", tag="ps_qkv")
        for i in range(2):
            p0 = i * 64
            mm(
                sc[:, i * QB:(i + 1) * QB],
                qk_sb[p0:p0 + 64, kcol + tk * 128: kcol + (tk + 1) * 128],
                qk_sb[p0:p0 + 64, qcol + tb * QB: qcol + (tb + 1) * QB],
                start=True,
                stop=True,
            )
        return sc

    # Schraudolph bf16 exp on the DVE for a few tiles per phase: bf16 bits of
    # exp(scale*s) ~= round(s*(scale*128/ln2) + (127*128 - 7.42)). The uniform
    # half-bit rounding bias cancels between numerator and denominator of the
    # softmax; residual noise is ~1.5% rms on the offloaded tiles.
    SCH_A = float(scale * 128.0 / np.log(2.0))
    SCH_B = float(127.0 * 128.0 - 7.42 + 0.5)
    SCH_SLOTS = (7, 9, 11, 13)

    def emit_exp_av(p, tk, sc):
        tb, g = divmod(p, 2)
        if tk in SCH_SLOTS:
            eti = exp_pool.tile([128, 2 * QB], mybir.dt.int16, name="et", tag="et")
            nc.vector.tensor_scalar(
                eti[:], sc[:], SCH_A, SCH_B,
                mybir.AluOpType.mult, mybir.AluOpType.add,
            )
            et = eti[:].bitcast(BF16)
        else:
            ett = exp_pool.tile([128, 2 * QB], BF16, name="et", tag="et")
            nc.scalar.activation(
                ett[:], sc[:], mybir.ActivationFunctionType.Exp, scale=scale
            )
            et = ett[:]
        for i in range(2):
            hh = 2 * g + i
            lhsT_v = v_sb[:, tk * HL * DE + hh * DE: tk * HL * DE + (hh + 1) * DE]
            mm(
                ypairs[p][i][0:DE, :],
                lhsT_v,
                et[:, i * QB:(i + 1) * QB],
                start=(tk == 0),
                stop=(tk == TT - 1),
            )

    norm_state = {}

    def emit_norm_step(p, step):
        """Normalization of phase p, split into steps 0..3 so the PE-side bc
        matmuls land well after the DVE-side reciprocals have finished."""
        tb, g = divmod(p, 2)
        yp = ypairs[p]
        if step == 0:
            recs = [small_pool.tile([1, QB], F32R, name="rec") for _ in range(2)]
            with nc.allow_low_precision(reason="f32r rounding for PE"):
                nc.vector.reciprocal(recs[0][:], yp[0][D:DE, :])
            norm_state[p] = [recs, None]
        elif step == 1:
            recs = norm_state[p][0]
            with nc.allow_low_precision(reason="f32r rounding for PE"):
                nc.vector.reciprocal(recs[1][:], yp[1][D:DE, :])
        elif step == 2:
            recs = norm_state[p][0]
            bcs = []
            for i in range(2):
                bc = main_ps.tile([64, QB], F32, name="bc", tag="ps_qkv")
                mm(
                    bc[:], sel_sb[0:1, 0:64], recs[i][:],
                    start=True, stop=True,
                )
                bc_sb = small_pool.tile([64, QB], F32, name="bc_sb")
                nc.vector.tensor_copy(bc_sb[:], bc[:])
                bcs.append(bc_sb)
            norm_state[p][1] = bcs
        else:
            bcs = norm_state[p][1]
            ycol = g * T + tb * QB
            for i in range(2):
                p0 = i * 64
                nc.vector.tensor_mul(
                    yT_sb[p0:p0 + 64, ycol: ycol + QB],
                    yp[i][0:D, :],
                    bcs[i][:],
                )
            del norm_state[p]

    cproj_state = {}

    def emit_cproj_step(tb, j):
        """c_proj output tile tt = tb*4 + j: both 512-wide halves (two
        consecutive PSUM allocs, keeping the sc ring parity), then DMA out."""
        tt = tb * (QB // 128) + j
        OSUB = 512
        osb = out_pool.tile([128, C], F32, name="osb")
        for ob in range(2):
            ops = main_ps.tile([128, OSUB], F32, name="ops", tag="ps_qkv")
            for kk in range(KK):
                lhsT_y = yT_sb[:, kk * T + tt * 128: kk * T + (tt + 1) * 128]
                mm(
                    ops[:],
                    lhsT_y,
                    wp_sb[:, kk * C + ob * OSUB: kk * C + (ob + 1) * OSUB],
                    start=(kk == 0),
                    stop=(kk == KK - 1),
                )
            nc.vector.tensor_copy(osb[:, ob * OSUB:(ob + 1) * OSUB], ops[:])
        nc.sync.dma_start(out[tt * 128:(tt + 1) * 128, :], osb[:])

    # norm steps for phase p-1 land at tk 0,1,6,8 of phase p (the two serial
    # 3.3us DVE reciprocals start immediately; the PE-side bc matmuls land 5
    # tk-slots later so they never head-of-line-block the PE queue); c_proj
    # for tb = p//2 - 1 lands at tk 9,11,13,15 of even phase p, one output
    # tile (= two consecutive PSUM allocs, keeping sc ring parity) per slot.
    NORM_SLOTS = (0, 1, 6, 8)
    CPROJ_SLOTS = (8, 10, 12, 14)

    sc_next = None
    for p in range(NPH):
        ypairs[p] = [y_ps.tile([128, QB], F32, name="yps") for _ in range(2)]
        if p == 0:
            sc_next = emit_qk(0, 0)
        for tk in range(TT):
            sc_cur = sc_next
            if tk < TT - 1:
                sc_next = emit_qk(p, tk + 1)
            elif p < NPH - 1:
                sc_next = emit_qk(p + 1, 0)
            else:
                sc_next = None
            if p == 0 and tk % 2 == 0 and tk <= 12:
                emit_vtrans(tk + 2)
                emit_vtrans(tk + 3)
            if p >= 1 and tk in NORM_SLOTS:
                emit_norm_step(p - 1, NORM_SLOTS.index(tk))
            if p >= 2 and p % 2 == 0 and tk in CPROJ_SLOTS:
                emit_cproj_step(p // 2 - 1, CPROJ_SLOTS.index(tk))
            emit_exp_av(p, tk, sc_cur)
        # free the previous phase's ypair tiles once fully consumed
    # tail: normalize the last phase and emit the last tb's c_proj
    for step in range(4):
        emit_norm_step(NPH - 1, step)
    for j in range(4):
        emit_cproj_step(NQB - 1, j)

    stack.close()


def build_nc(T=T_FULL, C=C_FULL):
    nc = bass.Bass("TRN2")
    CT = C // 128
    xT = nc.dram_tensor("xT", [C, T], BF16, kind="ExternalInput")
    wqk = nc.dram_tensor("wqk", [128, CT * 2 * CLOC], BF16, kind="ExternalInput")
    wv = nc.dram_tensor("wv", [128, CT * CLOC], BF16, kind="ExternalInput")
    wp = nc.dram_tensor("wp", [128, (CLOC // 128) * C], BF16, kind="ExternalInput")
    sel = nc.dram_tensor("sel", [2, 128], F32R, kind="ExternalInput")
    ones = nc.dram_tensor("ones", [128, (T // 128) * HL], BF16, kind="ExternalInput")
    ident = nc.dram_tensor("ident", [128, 128], BF16, kind="ExternalInput")
    out = nc.dram_tensor("out", [T, C], F32, kind="ExternalOutput")
    with tile.TileContext(nc) as tc:
        emit_mha_kernel(tc, out[:], xT[:], wqk[:], wv[:], wp[:], sel[:], ones[:], ident[:], T, C)
    return legalize_waits(nc)


def _sbuf_tiled(w):
    """[K, F] -> [128, (K//128)*F] with per-128-row chunks laid side by side
    (the layout emit_mha_kernel indexes as [p, ct*F + f])."""
    K, F = w.shape
    CT = K // 128
    return np.ascontiguousarray(
        w.reshape(CT, 128, F).transpose(1, 0, 2).reshape(128, CT * F)
    )


def make_in_maps(x, W_attn, W_proj):
    """Host-side shard + layout prep for the 8 cores."""
    bf16 = mybir.dt.np(BF16)
    C = x.shape[2]
    sel = np.zeros((2, 128), np.float32)
    sel[0, 0:64] = 1.0
    sel[1, 64:128] = 1.0
    in_maps = []
    for core in range(N_CORES):
        b, hg = divmod(core, N_CORES // B)
        s0, s1 = hg * CLOC, (hg + 1) * CLOC
        Wq = W_attn[s0:s1, :]
        Wk = W_attn[C + s0:C + s1, :]
        Wv = W_attn[2 * C + s0:2 * C + s1, :]
        in_maps.append({
            "sel": sel,
            "ident": np.eye(128).astype(bf16),
            "ones": np.ones((128, (x.shape[1] // 128) * HL), dtype=bf16),
            "xT": np.ascontiguousarray(x[b].T).astype(bf16),
            "wqk": _sbuf_tiled(np.concatenate([Wq, Wk], 0).T).astype(bf16),
            "wv": _sbuf_tiled(Wv.T).astype(bf16),
            "wp": _sbuf_tiled(W_proj[:, s0:s1].T).astype(bf16),
        })
    return in_maps


_CACHED_NC = None


def kernel(x, W_attn, W_proj, b_proj, _trace=False):
    global _CACHED_NC
    x = np.asarray(x, dtype=np.float32)
    W_attn = np.asarray(W_attn, dtype=np.float32)
    W_proj = np.asarray(W_proj, dtype=np.float32)
    b_proj = np.asarray(b_proj, dtype=np.float32)

    if _CACHED_NC is None:
        _CACHED_NC = build_nc(T=x.shape[1], C=x.shape[2])
    nc = _CACHED_NC

    in_maps = make_in_maps(x, W_attn, W_proj)
    res = bass_utils.run_bass_kernel_spmd(
        nc, in_maps, core_ids=list(range(N_CORES)), trace=_trace,
    )
    parts = [r["out"] for r in res.results]
    G = N_CORES // B
    out = np.stack(
        [np.sum(parts[b * G:(b + 1) * G], axis=0) + b_proj for b in range(B)], axis=0
    ).astype(np.float32)
    if _trace:
        return out, res
    return out


if __name__ == "__main__":
    nc = build_nc()
    print("built OK")
